# revision 7
# baseline (speedup 1.0000x reference)
"""Trainium2 Bass kernel for nn_Brain_connectomic_graph (GNN message passing).

Single tiny graph (N=100 nodes, E=2000 edges). The whole network is dense
linear algebra on ONE NeuronCore, replicated across the 8 cores (data-parallel
lanes with batch=1 per the sharding hint); core 0's output is returned.

All floating-point math runs on device; the host only does layout packing
(transposes/concats, int32 edge indices packed bit-exactly into the f32 blob,
pure 0/1 mask constants).

Key structure (v2, restructured from the first working version):
  - one-hot edge matrices built from int32 iota compares (contiguous writes),
    contracted on the PE into [A1 | Ag] in one PSUM accumulation,
  - degrees via ones-column matmuls; dis = 1/sqrt on Scalar+Vector,
  - GCN layers: hemisphere masks pre-applied off-path; bias added inside the
    PSUM accumulation via tiny rank-2 matmuls; epilogue = 2 DVE ops,
  - score computed in BOTH orientations by operand-swapped matmuls (row and
    column), with the rank matrix diagonal suppressed by accumulating
    -1e30*I into srep so the two orientations never have to be bit-identical,
  - ChebConv without any PE transposes: N = -D^-1/2 Atil D^-1/2 built once,
    then Tx1 = mm(N, h2), Tx1^T = mm(h2, N), (M~ Tx1)^T = mm(Tx1, N);
    s_raw = h2@(W0-W2) + Tx1@W1 + (M~Tx1)@(2*W2) + bc (all PSUM-accumulated),
  - diff-pool epilogue via mm(ex1, gat_r) / mm(ex2, h2*rc2) operand swaps,
  - ACT table loads: sqrt set in prologue, exp/tanh set prefetched by a dummy
    Exp right after the last Sqrt so no load sits on the critical path.
"""

import numpy as np

N = 100
E = 2000
EP = 2048          # padded edges: 16 chunks x 128 partitions
NCH = 16
K1 = 50

# ---- inbuf column layout (f32 blob [128, C]) --------------------------------
_off = 0
def _nxt(w):
    global _off
    o = _off
    _off += w
    return o

# DMA group A: edge data (src/dst packed as int32 bits in the f32 blob)
O_SRC   = _nxt(16)    # [128,16] src int32 (pad -1)
O_DST   = _nxt(16)    # [128,16] dst int32 (pad -1)
O_EW    = _nxt(16)    # [128,16] edge_attr f32 (pad 0)
C_DMA_A = _off
# DMA group B: first matmul operands
O_XT    = _nxt(100)   # [100,100] x^T
O_W1    = _nxt(128)   # [100,128] [Wl1 | Wr1]
C_DMA_B = _off
# DMA group C: everything else
O_W2    = _nxt(40)    # [64,40]   [Wl2 | Wr2]
O_WG    = _nxt(20)    # [20,20]   Wg1
O_WRR2  = _nxt(2)     # [20,2]    [Wrel | Wroot]
O_WC0   = _nxt(20)    # [20,20]   Wc0
O_WC1   = _nxt(20)    # [20,20]   Wc1
O_WC2   = _nxt(20)    # [20,20]   Wc2
O_MK2   = _nxt(100)   # [2,100]   rows: p0 = left mask, p1 = right mask
O_B12   = _nxt(64)    # [2,64]    rows: p0 = bl1, p1 = br1
O_B22   = _nxt(20)    # [2,20]    rows: p0 = bl2, p1 = br2
O_BG    = _nxt(20)    # [1,20]    bg1 row
O_BC    = _nxt(20)    # [1,20]    bc row
O_ONE1  = _nxt(100)   # [1,100]   ones row
O_ONESC = _nxt(1)     # [100,1]   ones column
O_MKL   = _nxt(1)     # [100,1]   left mask column
O_MKR   = _nxt(1)     # [100,1]   right mask column
O_MBD   = _nxt(100)   # [100,100] block mask: [b,a]=1 iff (b<50)==(a<50)
O_BREL  = _nxt(1)     # [128,1]   brel broadcast
C_COLS  = _off


def _split_multiwaits(bir: dict) -> dict:
    """This container's walrus accepts only ONE sync-wait per instruction.
    Insert single-wait NoOps (same engine, just before) for the extras."""
    for f in bir.get("functions", []):
        for bb in f.get("blocks", []):
            out = []
            for ins in bb.get("instructions", []):
                si = ins.get("sync_info")
                waits = (si or {}).get("on_wait") or []
                if len(waits) > 1:
                    for i, w in enumerate(waits[:-1]):
                        out.append({
                            "debug": ins.get("debug", 0),
                            "engine": ins["engine"],
                            "ins": [], "outs": [],
                            "name": f"{ins['name']}-w{i}",
                            "opcode": "NoOp",
                            "sync_info": {"on_wait": [w], "on_update": []},
                        })
                    si["on_wait"] = [waits[-1]]
                out.append(ins)
            bb["instructions"] = out
    return bir


def _build():
    import concourse.bass as bass
    import concourse.mybir as mybir
    import concourse.tile as tile

    f32 = mybir.dt.float32
    i32 = mybir.dt.int32
    Alu = mybir.AluOpType
    Act = mybir.ActivationFunctionType

    nc = bass.Bass("TRN2")
    in_a = nc.dram_tensor("inbufA", [128, C_DMA_A], f32, kind="ExternalInput")
    in_b = nc.dram_tensor("inbufB", [128, C_DMA_B - C_DMA_A], f32, kind="ExternalInput")
    in_c = nc.dram_tensor("inbufC", [128, C_COLS - C_DMA_B], f32, kind="ExternalInput")
    out_d = nc.dram_tensor("out", [K1, 20], f32, kind="ExternalOutput")

    with tile.TileContext(nc) as tc:
        with (
            tc.tile_pool(name="sb", bufs=1) as sb,
            tc.tile_pool(name="ps", bufs=1, space="PSUM") as ps,
        ):
            ib = sb.tile([128, C_COLS], f32, tag="ib", name="ib")
            nc.sync.dma_start(out=ib[:, 0:C_DMA_A], in_=in_a.ap())
            nc.sync.dma_start(out=ib[:, C_DMA_A:C_DMA_B], in_=in_b.ap())
            nc.sync.dma_start(out=ib[:, C_DMA_B:C_COLS], in_=in_c.ap())

            def isl(off, w, p0=0, p1=128):
                return ib[p0:p1, off:off + w]

            V = nc.vector
            S = nc.scalar
            P = nc.gpsimd
            T = nc.tensor
            mm = lambda shape, name, bufs=6: ps.tile(shape, f32, tag="mm", name=name, bufs=bufs)

            # ---- on-device constants (GpSimd), issued in need-order:
            # ones (PE warmups wait on it), iota (one-hots), identity (layers),
            # then everything needed only after the top-k.
            ones_t = sb.tile([128, 100], f32, tag="ones_t", name="ones_t")
            P.memset(ones_t, 1.0)
            iota_i = sb.tile([128, 100], i32, tag="iota_i", name="iota_i")
            P.iota(iota_i, pattern=[[1, 100]], base=0, channel_multiplier=0)
            i100_t = sb.tile([100, 100], f32, tag="i100_t", name="i100_t")
            P.memset(i100_t, 0.0)
            P.affine_select(out=i100_t, in_=i100_t, compare_op=Alu.not_equal,
                            fill=1.0, base=0, pattern=[[-1, 100]], channel_multiplier=1)
            ineg_t = sb.tile([100, 100], f32, tag="ineg_t", name="ineg_t")
            P.memset(ineg_t, 0.0)
            P.affine_select(out=ineg_t, in_=ineg_t, compare_op=Alu.not_equal,
                            fill=-1e30, base=0, pattern=[[-1, 100]], channel_multiplier=1)
            tril_t = sb.tile([100, 100], f32, tag="tril_t", name="tril_t")
            P.memset(tril_t, 1.0)
            P.affine_select(out=tril_t, in_=tril_t, compare_op=Alu.is_gt,
                            fill=0.0, base=0, pattern=[[-1, 100]], channel_multiplier=1)
            triu_t = sb.tile([100, 100], f32, tag="triu_t", name="triu_t")
            P.memset(triu_t, 1.0)
            P.affine_select(out=triu_t, in_=triu_t, compare_op=Alu.is_gt,
                            fill=0.0, base=0, pattern=[[1, 100]], channel_multiplier=-1)
            iota_t = sb.tile([128, 100], f32, tag="iota_t", name="iota_t")
            P.tensor_copy(out=iota_t, in_=iota_i)

            XT    = isl(O_XT, 100, 0, 100)
            SRCi  = isl(O_SRC, 16).bitcast(i32)
            DSTi  = isl(O_DST, 16).bitcast(i32)
            EW    = isl(O_EW, 16)
            W1    = isl(O_W1, 128, 0, 100)
            W2    = isl(O_W2, 40, 0, 64)
            WG    = isl(O_WG, 20, 0, 20)
            WRR2  = isl(O_WRR2, 2, 0, 20)
            WC0   = isl(O_WC0, 20, 0, 20)
            WC1   = isl(O_WC1, 20, 0, 20)
            WC2   = isl(O_WC2, 20, 0, 20)
            MK2   = isl(O_MK2, 100, 0, 2)
            B12   = isl(O_B12, 64, 0, 2)
            B22   = isl(O_B22, 20, 0, 2)
            BGr   = isl(O_BG, 20, 0, 1)
            BCr   = isl(O_BC, 20, 0, 1)
            ONE1  = isl(O_ONE1, 100, 0, 1)
            ONESC = isl(O_ONESC, 1, 0, 100)
            MKLc  = isl(O_MKL, 1, 0, 100)
            MKRc  = isl(O_MKR, 1, 0, 100)
            MBD   = isl(O_MBD, 100, 0, 100)
            BREL  = isl(O_BREL, 1)
            IO50  = iota_t[0:100, 0:50]
            I100  = i100_t[:, :]

            # weight combos for the Cheb restructure (off critical path)
            wc02 = sb.tile([20, 20], f32, tag="wc02", name="wc02")
            P.tensor_tensor(out=wc02, in0=WC0, in1=WC2, op=Alu.subtract)
            wc2x2 = sb.tile([20, 20], f32, tag="wc2x2", name="wc2x2")
            P.tensor_scalar(out=wc2x2, in0=WC2, scalar1=2.0, scalar2=None, op0=Alu.mult)

            # ---- ACT table prewarm: sqrt set resident for the early chain ---
            scr = sb.tile([1, 1], f32, tag="scr", name="scr")
            V.memset(scr, 1.0)
            S.activation(out=scr, in_=scr, func=Act.Sqrt)

            # ---- PE warmup (HAM) then xw = x @ [Wl1|Wr1] --------------------
            a_ps = ps.tile([100, 200], f32, tag="acc", name="a_ps", bufs=1)
            ones_w = ones_t[:, 0:100].unsqueeze(1).broadcast_to([128, 2, 100])
            for _ in range(4):
                T.matmul(a_ps, ones_t[:, :], ones_w)
            xw = mm([100, 128], "xw")
            T.matmul(xw, XT, W1)

            # ---- one-hot edge matrices, 4 chunk-groups, contiguous writes ---
            # ssrc[p, c*100+n] = [src==n]; sboth = [Sdst | Sdst*ew] stacked.
            ssrc = sb.tile([128, NCH * 100], f32, tag="ssrc", name="ssrc")
            sboth = sb.tile([128, 2 * NCH * 100], f32, tag="sboth", name="sboth")
            sb2 = sboth.rearrange("p (t c j) -> p t c j", t=2, c=NCH)
            ssrc3 = ssrc.rearrange("p (c j) -> p c j", c=NCH)
            a_ps3 = a_ps.rearrange("p (t j) -> p t j", t=2)
            GRP = 4
            for g in range(0, NCH, GRP):
                gs_, ge_ = g, g + GRP
                iota_b = iota_i.unsqueeze(1).broadcast_to([128, GRP, 100])
                src_b = SRCi[:, gs_:ge_].unsqueeze(2).broadcast_to([128, GRP, 100])
                dst_b = DSTi[:, gs_:ge_].unsqueeze(2).broadcast_to([128, GRP, 100])
                ew_b = EW[:, gs_:ge_].unsqueeze(2).broadcast_to([128, GRP, 100])
                V.tensor_tensor(out=sb2[:, 0, gs_:ge_, :], in0=iota_b, in1=dst_b, op=Alu.is_equal)
                V.tensor_tensor(out=ssrc3[:, gs_:ge_, :], in0=iota_b, in1=src_b, op=Alu.is_equal)
                P.tensor_tensor(out=sb2[:, 1, gs_:ge_, :], in0=sb2[:, 0, gs_:ge_, :], in1=ew_b, op=Alu.mult)
                for c in range(gs_, ge_):
                    T.matmul(a_ps3, ssrc3[:, c, :], sb2[:, :, c, :],
                             start=(c == 0), stop=(c == NCH - 1))

            # ---- adjacency post: agt = Ag + I, act = agt*MBD, A1 copy -------
            agt = sb.tile([100, 100], f32, tag="agt", name="agt")
            V.tensor_tensor(out=agt, in0=a_ps[:, 100:200], in1=I100, op=Alu.add)
            act = sb.tile([100, 100], f32, tag="act", name="act")
            V.tensor_tensor(out=act, in0=agt, in1=MBD, op=Alu.mult)
            # ---- degrees (column sums) + dis = 1/sqrt -----------------------
            ddp = mm([100, 2], "ddp")
            T.matmul(ddp[:, 0:1], act, ONESC)
            T.matmul(ddp[:, 1:2], agt, ONESC)
            dsq = sb.tile([100, 2], f32, tag="dsq", name="dsq")
            S.activation(out=dsq, in_=ddp, func=Act.Sqrt)
            a1t = sb.tile([100, 100], f32, tag="a1t", name="a1t")
            S.activation(out=a1t, in_=a_ps[:, 0:100], func=Act.Copy)
            dis2 = sb.tile([100, 2], f32, tag="dis2", name="dis2")
            V.reciprocal(out=dis2, in_=dsq)
            disc = dis2[:, 0:1]
            disg = dis2[:, 1:2]

            # ---- layer 1 ----------------------------------------------------
            # hemisphere select runs before dis is ready (off critical path)
            y1m = sb.tile([100, 64], f32, tag="y1m", name="y1m")
            V.tensor_scalar_mul(y1m, xw[:, 64:128], MKRc)
            V.scalar_tensor_tensor(out=y1m, in0=xw[:, 0:64], scalar=MKLc, in1=y1m,
                                   op0=Alu.mult, op1=Alu.add)
            y1 = sb.tile([100, 64], f32, tag="y1", name="y1")
            V.tensor_scalar_mul(y1, y1m, disc)
            # masked dis columns for layer 2 (off critical path once disc ready)
            dvl = sb.tile([100, 1], f32, tag="dvl", name="dvl")
            V.tensor_scalar_mul(dvl, disc, MKLc)
            dvr = sb.tile([100, 1], f32, tag="dvr", name="dvr")
            V.tensor_scalar_mul(dvr, disc, MKRc)
            z1 = mm([100, 64], "z1")
            T.matmul(z1, act, y1, start=True, stop=False)
            T.matmul(z1, MK2, B12, start=False, stop=True)
            h1 = sb.tile([100, 64], f32, tag="h1", name="h1")
            V.tensor_scalar_mul(h1, z1, disc)
            V.scalar_tensor_tensor(out=h1, in0=h1, scalar=0.01, in1=h1, op0=Alu.mult, op1=Alu.max)

            # ---- layer 2 ----------------------------------------------------
            h1t_p = mm([64, 100], "h1t_p")
            T.transpose(h1t_p, h1, I100)
            h1t = sb.tile([64, 100], f32, tag="h1t", name="h1t")
            V.tensor_copy(out=h1t, in_=h1t_p)
            xw2 = mm([100, 40], "xw2")
            T.matmul(xw2, h1t, W2)
            y2 = sb.tile([100, 20], f32, tag="y2", name="y2")
            V.tensor_scalar_mul(y2, xw2[:, 20:40], dvr)
            V.scalar_tensor_tensor(out=y2, in0=xw2[:, 0:20], scalar=dvl, in1=y2,
                                   op0=Alu.mult, op1=Alu.add)
            z2 = mm([100, 20], "z2")
            T.matmul(z2, act, y2, start=True, stop=False)
            T.matmul(z2, MK2, B22, start=False, stop=True)
            h2a = sb.tile([100, 20], f32, tag="h2a", name="h2a")
            V.tensor_scalar_mul(h2a, z2, disc)
            V.scalar_tensor_tensor(out=h2a, in0=h2a, scalar=0.01, in1=h2a, op0=Alu.mult, op1=Alu.max)

            # ---- global GCN layer ------------------------------------------
            h2at_p = mm([20, 100], "h2at_p")
            T.transpose(h2at_p, h2a, I100)
            h2at = sb.tile([20, 100], f32, tag="h2at", name="h2at")
            V.tensor_copy(out=h2at, in_=h2at_p)
            xwg = mm([100, 20], "xwg")
            T.matmul(xwg, h2at, WG)
            yg = sb.tile([100, 20], f32, tag="yg", name="yg")
            V.tensor_scalar_mul(yg, xwg, disg)
            zg = mm([100, 20], "zg")
            T.matmul(zg, agt, yg, start=True, stop=False)
            T.matmul(zg, ONE1, BGr, start=False, stop=True)
            # h2 lives in cols 0:20 of h2x; SAG score joins as col 20
            h2x = sb.tile([100, 21], f32, tag="h2x", name="h2x")
            h2 = h2x[:, 0:20]
            score = h2x[:, 20:21]
            V.tensor_scalar_mul(h2, zg, disg)
            V.scalar_tensor_tensor(out=h2, in0=h2, scalar=0.01, in1=h2, op0=Alu.mult, op1=Alu.max)
            h2t_p = mm([20, 100], "h2t_p")
            T.transpose(h2t_p, h2, I100)
            h2t = sb.tile([20, 100], f32, tag="h2t", name="h2t")
            V.tensor_copy(out=h2t, in_=h2t_p)

            # ---- SAGPool score in BOTH orientations -------------------------
            hw = mm([100, 2], "hw")
            T.matmul(hw, h2t, WRR2)
            hw_sb = sb.tile([100, 2], f32, tag="hw_sb", name="hw_sb")
            V.tensor_copy(out=hw_sb, in_=hw)
            sc_p = mm([100, 1], "sc_p")
            T.matmul(sc_p, a1t, hw_sb[:, 0:1], start=True, stop=False)
            T.matmul(sc_p, I100, hw_sb[:, 1:2], start=False, stop=True)
            srow_p = mm([1, 100], "srow_p")
            T.matmul(srow_p, hw_sb[:, 0:1], a1t, start=True, stop=False)
            T.matmul(srow_p, hw_sb[:, 1:2], I100, start=False, stop=True)
            V.tensor_copy(out=score, in_=sc_p)
            srow = sb.tile([1, 100], f32, tag="srow", name="srow")
            V.tensor_copy(out=srow, in_=srow_p)

            # ---- rank / top-k (diagonal suppressed via -1e30*I) -------------
            srep = ps.tile([100, 100], f32, tag="rep", name="srep", bufs=1)
            T.matmul(srep, ONE1, srow, start=True, stop=False)
            T.matmul(srep, ineg_t, I100, start=False, stop=True)
            t2 = sb.tile([100, 100], f32, tag="t2", name="t2")
            V.scalar_tensor_tensor(out=t2, in0=srep, scalar=score, in1=tril_t,
                                   op0=Alu.is_equal, op1=Alu.mult)
            csum = sb.tile([100, 100], f32, tag="csum", name="csum")
            rank = sb.tile([100, 1], f32, tag="rank", name="rank")
            V.scalar_tensor_tensor(out=csum, in0=srep, scalar=score, in1=t2,
                                   op0=Alu.is_gt, op1=Alu.add, accum_out=rank)
            kept = sb.tile([100, 1], f32, tag="kept", name="kept")
            V.tensor_scalar(out=kept, in0=rank, scalar1=49.5, scalar2=None, op0=Alu.is_lt)
            pit = sb.tile([100, 50], f32, tag="pit", name="pit")
            V.tensor_scalar(out=pit, in0=IO50, scalar1=rank, scalar2=None, op0=Alu.is_equal)
            srank_p = mm([100, 1], "srank_p")
            T.matmul(srank_p, triu_t, kept)
            gat = sb.tile([100, 50], f32, tag="gat", name="gat")
            V.scalar_tensor_tensor(out=gat, in0=IO50, scalar=srank_p,
                                   in1=kept.broadcast_to([100, 50]),
                                   op0=Alu.is_equal, op1=Alu.mult)

            # ---- pooled rows (overlaps the Cheb section) --------------------
            p1 = mm([50, 21], "p1")
            T.matmul(p1, pit, h2x[:, 0:21])
            th = sb.tile([50, 1], f32, tag="th", name="th")
            p1s = sb.tile([50, 20], f32, tag="p1s", name="p1s")

            # ---- pooled adjacency + disch -----------------------------------
            m1 = mm([100, 50], "m1")
            T.matmul(m1, a1t, pit)
            m1s = sb.tile([100, 50], f32, tag="m1s", name="m1s")
            S.activation(out=m1s, in_=m1, func=Act.Copy)
            at_p = mm([50, 50], "at_p")
            T.matmul(at_p, m1s, pit)          # Atil = P A1 P^T
            atil = sb.tile([50, 50], f32, tag="atil", name="atil")
            V.tensor_copy(out=atil, in_=at_p)
            u_p = mm([100, 1], "u_p")
            T.matmul(u_p, a1t, kept)          # A1^T kept
            u_sb = sb.tile([100, 1], f32, tag="u_sb", name="u_sb")
            V.tensor_copy(out=u_sb, in_=u_p)
            degc = mm([50, 1], "degc")
            T.matmul(degc, pit, u_sb)         # P (A1^T kept) = colsums of Atil
            dm = sb.tile([50, 1], f32, tag="dm", name="dm")
            V.tensor_scalar(out=dm, in0=degc, scalar1=1e-12, scalar2=None, op0=Alu.max)
            m0 = sb.tile([50, 1], f32, tag="m0", name="m0")
            V.tensor_scalar(out=m0, in0=degc, scalar1=0.0, scalar2=None, op0=Alu.is_gt)
            sqc = sb.tile([50, 1], f32, tag="sqc", name="sqc")
            S.activation(out=sqc, in_=dm, func=Act.Sqrt)
            # prefetch the exp/tanh ACT table right after the last Sqrt
            S.activation(out=scr, in_=scr, func=Act.Exp)
            rq = sb.tile([50, 1], f32, tag="rq", name="rq")
            V.reciprocal(out=rq, in_=sqc)
            disch = sb.tile([50, 1], f32, tag="disch", name="disch")
            V.tensor_tensor(out=disch, in0=rq, in1=m0, op=Alu.mult)
            ndisch = sb.tile([50, 1], f32, tag="ndisch", name="ndisch")
            V.tensor_scalar(out=ndisch, in0=rq, scalar1=m0, scalar2=-1.0, op0=Alu.mult, op1=Alu.mult)

            # ---- N = -D^-1/2 Atil D^-1/2 (as lhsT for all Cheb matmuls) -----
            drow_p = mm([1, 50], "drow_p")
            T.matmul(drow_p, ndisch, i100_t[0:50, 0:50])
            drow = sb.tile([1, 50], f32, tag="drow", name="drow")
            V.tensor_copy(out=drow, in_=drow_p)
            drep = mm([50, 50], "drep")
            T.matmul(drep, ONE1[0:1, 0:50], drow)   # rows = -disch^T
            nmat = sb.tile([50, 50], f32, tag="nmat", name="nmat")
            V.scalar_tensor_tensor(out=nmat, in0=atil, scalar=disch, in1=drep,
                                   op0=Alu.mult, op1=Alu.mult)

            # ---- Cheb Tx1 / (M~ Tx1), transpose-free ------------------------
            h2_50 = h2x[0:50, 0:20]
            tx1_p = mm([50, 20], "tx1_p")
            T.matmul(tx1_p, nmat, h2_50)      # Tx1 = M~ h2  (rows 0:50)
            tx1T_p = mm([20, 50], "tx1T_p")
            T.matmul(tx1T_p, h2_50, nmat)     # Tx1^T
            tx1 = sb.tile([50, 20], f32, tag="tx1", name="tx1")
            V.tensor_copy(out=tx1, in_=tx1_p)
            tx1T = sb.tile([20, 50], f32, tag="tx1T", name="tx1T")
            S.activation(out=tx1T, in_=tx1T_p, func=Act.Copy)
            S.activation(out=th, in_=p1[:, 20:21], func=Act.Tanh, bias=BREL[0:50, :], scale=1.0)
            S.activation(out=p1s, in_=p1[:, 0:20], func=Act.Copy)
            ttT_p = mm([20, 50], "ttT_p")
            T.matmul(ttT_p, tx1, nmat)        # (M~ Tx1)^T
            ttT = sb.tile([20, 50], f32, tag="ttT", name="ttT")
            V.tensor_copy(out=ttT, in_=ttT_p)

            # ---- s_raw = h2@(W0-W2) + bc + Tx1@W1 + (M~Tx1)@(2W2) -----------
            sraw = mm([100, 20], "sraw")
            T.matmul(sraw, h2t, wc02, start=True, stop=False)
            T.matmul(sraw[0:50, :], tx1T, WC1, start=False, stop=False, skip_group_check=True)
            T.matmul(sraw[0:50, :], ttT, wc2x2, start=False, stop=False, skip_group_check=True)
            T.matmul(sraw, ONE1, BCr, start=False, stop=True)

            # ---- double softmax (normalizations folded into consumers) ------
            ex1 = sb.tile([100, 20], f32, tag="ex1", name="ex1")
            sum1 = sb.tile([100, 1], f32, tag="sum1", name="sum1")
            S.activation(out=ex1, in_=sraw, func=Act.Exp, accum_out=sum1)
            rc1 = sb.tile([100, 1], f32, tag="rc1", name="rc1")
            V.reciprocal(out=rc1, in_=sum1)
            ex2 = sb.tile([100, 20], f32, tag="ex2", name="ex2")
            sum2 = sb.tile([100, 1], f32, tag="sum2", name="sum2")
            S.activation(out=ex2, in_=ex1, func=Act.Exp, scale=rc1, accum_out=sum2)
            rc2 = sb.tile([100, 1], f32, tag="rc2", name="rc2")
            V.reciprocal(out=rc2, in_=sum2)

            # ---- diff-pool + output (operand-swapped, no transposes) --------
            h2r = sb.tile([100, 20], f32, tag="h2r", name="h2r")
            V.tensor_scalar_mul(h2r, h2, rc2)
            gat_r = sb.tile([100, 50], f32, tag="gat_r", name="gat_r")
            P.tensor_scalar_mul(gat_r, gat, rc1)
            hc_p = mm([20, 20], "hc_p")
            T.matmul(hc_p, ex2, h2r)          # Hc = s2^T h2
            hc = sb.tile([20, 20], f32, tag="hc", name="hc")
            V.tensor_copy(out=hc, in_=hc_p)
            m1T_p = mm([20, 50], "m1T_p")
            T.matmul(m1T_p, ex1, gat_r)       # (Gamma_r^T E)^T = inter^T
            m1T = sb.tile([20, 50], f32, tag="m1T", name="m1T")
            V.tensor_copy(out=m1T, in_=m1T_p)
            gp = mm([50, 20], "gp")
            T.matmul(gp, m1T, hc)             # inter @ Hc (rows in perm order)
            outv = sb.tile([50, 20], f32, tag="outv", name="outv")
            V.scalar_tensor_tensor(out=outv, in0=p1s, scalar=th, in1=gp,
                                   op0=Alu.mult, op1=Alu.add)
            nc.sync.dma_start(out=out_d.ap(), in_=outv)

    # walrus single-wait workaround
    orig = nc.to_json_bytes
    def patched(*a, **k):
        import json as _json
        return _json.dumps(_split_multiwaits(_json.loads(orig(*a, **k)))).encode()
    nc.to_json_bytes = patched
    return nc


def _pack(inputs) -> np.ndarray:
    f = lambda k: np.asarray(inputs[k], dtype=np.float32)
    blob = np.zeros((128, C_COLS), dtype=np.float32)

    x = f("x")
    blob[0:100, O_XT:O_XT + 100] = x.T

    ei = np.asarray(inputs["edge_index"]).astype(np.int64)
    # pad = INT_MIN: int32 bits 0x80000000 == float -0.0 (the f32 blob must
    # not contain NaN bit patterns), never equal to any iota value 0..99
    src = np.full(EP, -2**31, np.int32); src[:E] = ei[0].astype(np.int32)
    dst = np.full(EP, -2**31, np.int32); dst[:E] = ei[1].astype(np.int32)
    ew = np.zeros(EP, np.float32); ew[:E] = f("edge_attr")
    # column-chunk layout: element (p, c) = edge c*128+p; int32 packed bitwise
    blob[:, O_SRC:O_SRC + 16] = src.reshape(NCH, 128).T.view(np.float32)
    blob[:, O_DST:O_DST + 16] = dst.reshape(NCH, 128).T.view(np.float32)
    blob[:, O_EW:O_EW + 16] = ew.reshape(NCH, 128).T

    blob[0:100, O_W1:O_W1 + 64] = f("Wl1")
    blob[0:100, O_W1 + 64:O_W1 + 128] = f("Wr1")
    blob[0:64, O_W2:O_W2 + 20] = f("Wl2")
    blob[0:64, O_W2 + 20:O_W2 + 40] = f("Wr2")
    blob[0:20, O_WG:O_WG + 20] = f("Wg1")
    blob[0:20, O_WRR2] = f("Wrel")[:, 0]
    blob[0:20, O_WRR2 + 1] = f("Wroot")[:, 0]
    blob[0:20, O_WC0:O_WC0 + 20] = f("Wc0")
    blob[0:20, O_WC1:O_WC1 + 20] = f("Wc1")
    blob[0:20, O_WC2:O_WC2 + 20] = f("Wc2")
    half = np.arange(100) < 50
    blob[0, O_MK2:O_MK2 + 100] = half.astype(np.float32)
    blob[1, O_MK2:O_MK2 + 100] = (~half).astype(np.float32)
    blob[0, O_B12:O_B12 + 64] = f("bl1")
    blob[1, O_B12:O_B12 + 64] = f("br1")
    blob[0, O_B22:O_B22 + 20] = f("bl2")
    blob[1, O_B22:O_B22 + 20] = f("br2")
    blob[0, O_BG:O_BG + 20] = f("bg1")
    blob[0, O_BC:O_BC + 20] = f("bc")
    blob[0, O_ONE1:O_ONE1 + 100] = 1.0
    blob[0:100, O_ONESC] = 1.0
    blob[0:100, O_MKL] = half.astype(np.float32)
    blob[0:100, O_MKR] = (~half).astype(np.float32)
    blob[0:100, O_MBD:O_MBD + 100] = (half[:, None] == half[None, :]).astype(np.float32)
    blob[:, O_BREL] = f("brel")[0]
    return blob


_NC = None

def _get_nc():
    global _NC
    if _NC is None:
        _NC = _build()
    return _NC


def run(inputs, trace=False):
    from concourse.bass_utils import run_bass_kernel_spmd
    nc = _get_nc()
    blob = _pack(inputs)
    parts = {
        "inbufA": np.ascontiguousarray(blob[:, 0:C_DMA_A]),
        "inbufB": np.ascontiguousarray(blob[:, C_DMA_A:C_DMA_B]),
        "inbufC": np.ascontiguousarray(blob[:, C_DMA_B:C_COLS]),
    }
    in_maps = [dict(parts) for _ in range(8)]
    res = run_bass_kernel_spmd(nc, in_maps, list(range(8)), trace=trace)
    out = np.asarray(res.results[0]["out"], dtype=np.float32).reshape(1, K1 * 20)
    return out, res


def kernel(**inputs) -> np.ndarray:
    out, _ = run(inputs)
    return out
